# revision 1
# baseline (speedup 1.0000x reference)
"""Trainium2 Bass kernel for PVT-style spatial-reduction attention.

Reference computation (per batch):
  q = x @ q_w + q_b                               [4096, 320], 5 heads x 64
  x_ = conv2x2_stride2(x as [64,64,320], sr_w) + sr_b   -> [1024, 320]
  x_ = LayerNorm(x_) * ln_g + ln_b
  kv = x_ @ kv_w + kv_b -> k, v  [1024, 320] each
  out = softmax(q k^T / 8) v  per head -> [4096, 320]
  out = out @ proj_w + proj_b

Sharding: pure data parallelism, batch i -> core i (B == n_cores == 8).
No collectives.

Layout: channels on partitions, tokens on the free dim ("T-layout");
x is PE-transposed into patch-major xP so conv and q-proj are plain
matmuls; biases ride an appended ones-row; LN gamma/beta fold into
kv_w/kv_b on the host.

Perf notes (measured on HW, p50/min of repeat-129 loop diffs):
  - The fp32r baseline (~320us/iter) was PE-bound: fp32r matmuls
    self-load their stationary operand (~107ns per 128-col weight
    tile, not pipelinable - fp32r cannot use explicit LDWEIGHTS).
    Converting the scores (kT, qT), conv (xP, srw), q-proj (qw),
    out-proj (attnT, pw), PV (v_all, p_t), kv (xnT, kw, vw) and
    division (e5, rec5) operands to bf16 lets walrus emit separate
    LDWEIGHTS instructions that the PE's 64-deep reorder window pulls
    ahead of in-flight matmuls, hiding the weight loads
    (rel err 5.1e-3, gate 2e-2). PSUM accumulation stays fp32
    throughout; LN statistics and the softmax denominator sums are
    exact. The PE transposes stay f32r (bf16 transpose wedges the
    device); the psD->xnT scalar.copy does the bf16 cast instead.
  - Head 4's exp runs on DVE (int16-bf16 Schraudolph fast-exp) instead
    of ACT, trimming 32 of the 160 exp instructions off the bottleneck
    engine (rel err 9.0e-3 total, deterministic vs the fixed-seed
    reference).
  - Scores for head pairs (0,1)/(2,3) are adjacent row-tiled matmuls
    (rows 0-63 / 64-127, tile_position auto-derived) writing the two
    halves of a [128,1024] 2-bank PSUM tile, consumed by ONE batched
    ACT exp instruction. Head 4 batches consecutive key chunks.
  - The PV chain stays exact fp32r (v' carries a ones column whose PV
    row is the softmax denominator); division uses a K=5 head-select
    broadcast matmul on PE + DVE multiplies (a GPSIMD
    partition_broadcast variant measured +180us - do not revisit).
  - Deferred work (psO drains, division, q-proj, out-proj) sits in
    urgent/lazy FIFOs drained at fixed slots in the mc loops so the PE
    never takes a long serial burst between exp feeds.
  - PSUM: head phase uses a scoped 6-bank pool released before the
    attention pool (score pairs 2x2 banks + psO 2 + misc 2).
"""

import numpy as np

import concourse.bass as bass
import concourse.mybir as mybir
from concourse import bacc
from concourse.tile import TileContext

F32 = mybir.dt.float32
F32R = mybir.dt.float32r
BF16 = mybir.dt.bfloat16
I32 = mybir.dt.int32
I16 = mybir.dt.int16
AF = mybir.ActivationFunctionType
ALU = mybir.AluOpType
AX = mybir.AxisListType

B, N, C = 8, 4096, 320
HEAD, HD, SR = 5, 64, 2
NKV = 1024  # (64/2)*(64/2)
SCALE = HD ** -0.5
EPS = 1e-5
# Schraudolph fast-exp constants, bf16 variant:
# exp(x) ~ bitcast_bf16(int16(x * 2^7/ln2 + (127*128 - 5.59)))
FEXP_A = float(2 ** 7 / np.log(2))
FEXP_B = float(127 * 128 - 5.59)
N_CORES = 8

# channel chunking of C=320 into partitions: 128 + 128 + 64 (+1 ones row)
CK = [(0, 128), (128, 128), (256, 64)]


def _csz(k, aug):
    return CK[k][1] + (1 if (aug and k == 2) else 0)


def build_nc(repeat=1, loop_part="all", div_mode="pe", ablate=(), dve_exp=True, gp_mult=False):
    nc = bacc.Bacc()

    x = nc.declare_dram_parameter("x", [N, C], F32, isOutput=False)
    qw = nc.declare_dram_parameter("qw", [C + 1, C], BF16, isOutput=False)
    srw = nc.declare_dram_parameter("srw", [4 * C + 1, C], BF16, isOutput=False)
    kw = nc.declare_dram_parameter("kw", [C + 1, C], BF16, isOutput=False)
    vw = nc.declare_dram_parameter("vw", [C + 1, C], BF16, isOutput=False)
    pw = nc.declare_dram_parameter("pw", [C + 1, C], BF16, isOutput=False)
    ident = nc.declare_dram_parameter("ident", [128, 128], F32, isOutput=False)
    ones_c = nc.declare_dram_parameter("ones_c", [1, N], F32, isOutput=False)
    ones2 = nc.declare_dram_parameter("ones2", [128, 64], F32, isOutput=False)
    e5 = nc.declare_dram_parameter("e5", [HEAD, C], BF16, isOutput=False)
    out = nc.declare_dram_parameter("out", [N, C], F32, isOutput=True)

    import contextlib

    with TileContext(nc) as tc:
        with (
            nc.allow_low_precision(reason="float32r tiles are full fp32 storage"),
            tc.tile_pool(name="const", bufs=1) as cpool,
            tc.tile_pool(name="main", bufs=1) as mpool,
            tc.For_i(0, repeat, 1)
            if (repeat > 1 and loop_part == "all")
            else contextlib.nullcontext(),
        ):
            # ---- tile allocations ----------------------------------------
            id_sb = cpool.tile([128, 128], F32R)
            qw_sb = [
                cpool.tile([_csz(k, True), C], BF16, name=f"qw_sb{k}") for k in range(3)
            ]
            kw_sb = [
                cpool.tile([_csz(k, True), C], BF16, name=f"kw_sb{k}") for k in range(3)
            ]
            vw_sb = [
                cpool.tile([_csz(k, True), C], BF16, name=f"vw_sb{k}") for k in range(3)
            ]
            pw_sb = [
                cpool.tile([_csz(k, True), C], BF16, name=f"pw_sb{k}") for k in range(3)
            ]
            srw_sb = [
                [
                    cpool.tile([_csz(k, b == 0), C], BF16, name=f"srw_sb{b}{k}")
                    for k in range(3)
                ]
                for b in range(4)
            ]
            eps_sb = cpool.tile([128, 1], F32)
            e5_sb = cpool.tile([HEAD, C], BF16)

            xP = [
                [
                    mpool.tile([_csz(k, True), NKV], BF16, name=f"xP{b}{k}")
                    for k in range(3)
                ]
                for b in range(4)
            ]
            xnT = [
                mpool.tile([_csz(k, True), NKV], BF16, name=f"xnT{k}") for k in range(3)
            ]
            kT = [
                mpool.tile([_csz(k, False), NKV], BF16, name=f"kT{k}") for k in range(3)
            ]
            # v' [1024, 5*65] stored as [128, 8*325]; col t*325 + h*65 + d,
            # d==64 is the ones column (softmax denominator trick)
            v_all = mpool.tile([128, 8 * 325], BF16)

            # ============ head: x ingest + conv + LayerNorm + x_n^T =======
            with (
                tc.For_i(0, repeat, 1)
                if (repeat > 1 and loop_part == "head")
                else contextlib.nullcontext(),
                tc.tile_pool(name="px", bufs=1) as xpool,
                tc.tile_pool(name="hps", space="PSUM", bufs=1) as hpsp,
            ):

                def emit_window_ingest(g):
                    xa = []
                    for ii in range(4):
                        i = g * 4 + ii
                        t = xpool.tile([128, C], F32R, name="xa", tag="xa", bufs=6)
                        nc.sync.dma_start(
                            out=t[:], in_=x[i * 128 : (i + 1) * 128, :].bitcast(F32R)
                        )
                        xa.append(t)
                    for k in range(3):
                        csz = _csz(k, False)
                        psA = hpsp.tile(
                            [csz, 512], F32R, name="psA", tag="ps512", bufs=3
                        )
                        for ii in range(4):
                            nc.tensor.transpose(
                                psA[:, ii * 128 : (ii + 1) * 128],
                                xa[ii][:, CK[k][0] : CK[k][0] + csz],
                                id_sb[:],
                            )
                        # scatter the 512-token window into the 4 patch blocks
                        src = psA.rearrange("p (i a j c) -> p i a j c", i=4, a=2, j=32)
                        for b in range(4):
                            di, dj = b // 2, b % 2
                            dst = xP[b][k][0:csz, g * 128 : (g + 1) * 128].rearrange(
                                "p (i j) -> p i j", i=4
                            )
                            if g >= 4 or b % 2 == 0:
                                nc.vector.tensor_copy(dst, src[:, :, di, :, dj])
                            else:
                                nc.scalar.copy(dst, src[:, :, di, :, dj])

                def emit_conv_ln(t8):
                    psC = hpsp.tile([128, C], F32, name="psC", tag="ps320", bufs=3)
                    idx = 0
                    for b in range(4):
                        for k in range(3):
                            kp = _csz(k, b == 0)
                            nc.tensor.matmul(
                                psC[:],
                                xP[b][k][0:kp, t8 * 128 : (t8 + 1) * 128],
                                srw_sb[b][k][0:kp, :],
                                start=(idx == 0),
                                stop=(idx == 11),
                            )
                            idx += 1
                    sum_t = xpool.tile([128, 1], F32, name="sum_t", tag="st1", bufs=4)
                    negmu = xpool.tile([128, 1], F32, name="negmu", tag="st2", bufs=4)
                    vsum = xpool.tile([128, 1], F32, name="vsum", tag="st3", bufs=4)
                    sd = xpool.tile([128, 1], F32, name="sd", tag="st4", bufs=4)
                    inv = xpool.tile([128, 1], F32, name="inv", tag="st5", bufs=4)
                    scr = xpool.tile([128, C], F32, name="scr", tag="scr", bufs=3)
                    scr2 = xpool.tile([128, C], F32, name="scr2", tag="scr2", bufs=2)
                    xn = xpool.tile([128, C], F32R, name="xn", tag="xn", bufs=3)

                    nc.vector.tensor_reduce(sum_t[:], psC[:], axis=AX.X, op=ALU.add)
                    nc.vector.tensor_scalar_mul(negmu[:], sum_t[:], -1.0 / C)
                    nc.vector.tensor_scalar(
                        out=scr[:], in0=psC[:], scalar1=negmu[:], scalar2=None, op0=ALU.add
                    )
                    nc.vector.scalar_tensor_tensor(
                        out=scr2[:],
                        in0=scr[:],
                        scalar=0.0,
                        in1=scr[:],
                        op0=ALU.add,
                        op1=ALU.mult,
                        accum_out=vsum[:],
                    )
                    nc.scalar.activation(
                        sd[:], vsum[:], AF.Sqrt, scale=1.0 / C, bias=eps_sb[:]
                    )
                    nc.vector.reciprocal(inv[:], sd[:])
                    nc.vector.tensor_scalar_mul(xn[:], scr[:], inv[:])
                    # transpose x_n -> xnT
                    for k in range(3):
                        csz = _csz(k, False)
                        psD = hpsp.tile(
                            [csz, 128], F32R, name="psD", tag="ps512", bufs=3
                        )
                        nc.tensor.transpose(
                            psD[:], xn[:, CK[k][0] : CK[k][0] + csz], id_sb[:]
                        )
                        nc.scalar.copy(xnT[k][0:csz, t8 * 128 : (t8 + 1) * 128], psD[:])

                def emit_v(t8):
                    psF = hpsp.tile([128, C], F32, name="psF", tag="ps320", bufs=3)
                    for k in range(3):
                        kp = _csz(k, True)
                        nc.tensor.matmul(
                            psF[:],
                            xnT[k][0:kp, t8 * 128 : (t8 + 1) * 128],
                            vw_sb[k][0:kp, :],
                            start=(k == 0),
                            stop=(k == 2),
                        )
                    dst = v_all[:, t8 * 325 : (t8 + 1) * 325].rearrange(
                        "p (h e) -> p h e", h=5
                    )[:, :, 0:64]
                    nc.scalar.copy(dst, psF.rearrange("p (h d) -> p h d", h=5))

                def emit_kT(nn):
                    for m in range(3):
                        msz = _csz(m, False)
                        psE = hpsp.tile(
                            [msz, 512], F32, name="psE", tag="ps512", bufs=3
                        )
                        for k in range(3):
                            kp = _csz(k, True)
                            nc.tensor.matmul(
                                psE[:],
                                kw_sb[k][:, CK[m][0] : CK[m][0] + msz],
                                xnT[k][0:kp, nn * 512 : (nn + 1) * 512],
                                start=(k == 0),
                                stop=(k == 2),
                            )
                        nc.scalar.copy(kT[m][:, nn * 512 : (nn + 1) * 512], psE[:])

                nc.sync.dma_start(out=id_sb[:], in_=ident[:].bitcast(F32R))
                nc.vector.memset(eps_sb[:], EPS)
                nc.sync.dma_start(out=e5_sb[:], in_=e5[:])
                emit_window_ingest(0)
                for b in range(4):
                    base = 0 if b == 0 else 321 + (b - 1) * C
                    for k in range(3):
                        p = _csz(k, b == 0)
                        nc.sync.dma_start(
                            out=srw_sb[b][k][:],
                            in_=srw[base + CK[k][0] : base + CK[k][0] + p, :],
                        )
                for b in range(4):
                    nc.vector.memset(xP[b][2][64:65, :], 1.0)
                for k in range(3):
                    r0, p = CK[k][0], _csz(k, True)
                    nc.sync.dma_start(out=vw_sb[k][:], in_=vw[r0 : r0 + p, :])
                    nc.sync.dma_start(out=kw_sb[k][:], in_=kw[r0 : r0 + p, :])
                nc.vector.memset(xnT[2][64:65, :], 1.0)
                nc.vector.memset(
                    v_all.rearrange("p (t h e) -> p t h e", t=8, h=5)[:, :, :, 64], 1.0
                )

                for g in range(1, 8):
                    emit_window_ingest(g)
                    emit_conv_ln(g - 1)
                    if g >= 2:
                        emit_v(g - 2)
                    if g == 5:
                        emit_kT(0)
                    if g == 2:
                        for k in range(3):
                            r0, p = CK[k][0], _csz(k, True)
                            nc.sync.dma_start(
                                out=qw_sb[k][:], in_=qw[r0 : r0 + p, :]
                            )
                            nc.sync.dma_start(
                                out=pw_sb[k][:], in_=pw[r0 : r0 + p, :]
                            )
                emit_conv_ln(7)
                emit_v(6)
                emit_v(7)
                emit_kT(1)

            # ================= attention + q-proj + out-proj ==============
            with (
                tc.For_i(0, repeat, 1)
                if (repeat > 1 and loop_part == "att")
                else contextlib.nullcontext(),
                tc.tile_pool(name="att", bufs=1) as apool,
                tc.tile_pool(name="aps", space="PSUM", bufs=1) as apsp,
            ):
                attnT = [
                    apool.tile([_csz(k, True), N], BF16, name=f"attnT{k}")
                    for k in range(3)
                ]
                nc.vector.memset(attnT[2][64:65, :], 1.0)

                out_r = out.rearrange("(i a j c) d -> i a j c d", a=2, j=32, c=2)

                qhold = {}

                def emit_q_m(n, m):
                    # q^T chunk (n, m-rows) -> rotating tile; 3 matmuls
                    b, nn = n // 2, n % 2
                    msz = _csz(m, False)
                    psB = apsp.tile([msz, 512], F32, name="psB", tag="ps320", bufs=2)
                    for k in range(3):
                        kp = _csz(k, True)
                        nc.tensor.matmul(
                            psB[:],
                            qw_sb[k][:, CK[m][0] : CK[m][0] + msz],
                            xP[b][k][0:kp, nn * 512 : (nn + 1) * 512],
                            start=(k == 0),
                            stop=(k == 2),
                        )
                    qtn = apool.tile(
                        [msz, 512], BF16, name=f"qTn{m}", tag=f"qtn{m}", bufs=2
                    )
                    nc.vector.tensor_copy(qtn[:], psB[:])
                    qhold.setdefault(n, [None, None, None])[m] = qtn

                recbs = {}
                se_all = apool.tile([1, HEAD * 512], F32, name="se_all") if div_mode == "pe" else None
                se5s = {}

                def emit_division_s1(n, h, psO):
                    # drain psO: unnormalized out -> attnT + stash sumexp row
                    hc, hr = h // 2, (h % 2) * 64
                    ns = slice(n * 512, (n + 1) * 512)
                    nc.vector.tensor_copy(attnT[hc][hr : hr + 64, ns], psO[0:64, :])
                    if "div" in ablate:
                        return
                    if div_mode == "pe":
                        nc.vector.tensor_copy(
                            se_all[0:1, h * 512 : (h + 1) * 512], psO[64:65, :]
                        )
                        if h == HEAD - 1:
                            se5 = apool.tile(
                                [HEAD, 512], F32, name="se5", tag="se5", bufs=2
                            )
                            nc.sync.dma_start(out=se5[:], in_=se_all[0:1, :])
                            se5s[n] = se5
                        return
                    rec = apool.tile([1, 512], F32R, name="rec", tag="rec", bufs=4)
                    nc.vector.reciprocal(rec[:], psO[64:65, :])
                    recb = apool.tile([128, 512], F32R, name="recb", tag="recb", bufs=4)
                    nc.gpsimd.partition_broadcast(recb[:], rec[0:1, :])
                    recbs[(n, h)] = recb

                def emit_division_s2(n, h):
                    if "div" in ablate:
                        return
                    ns = slice(n * 512, (n + 1) * 512)
                    if div_mode == "pe":
                        # batched per n: one reciprocal, K=5 head-select
                        # broadcast matmul, 3 in-place multiplies
                        rec5 = apool.tile(
                            [HEAD, 512], BF16, name="rec5", tag="rec5", bufs=2
                        )
                        nc.vector.reciprocal(rec5[:], se5s.pop(n)[:])
                        for k in range(3):
                            csz = _csz(k, False)
                            psR = apsp.tile(
                                [csz, 512], F32, name="psR", tag="ps320", bufs=2
                            )
                            nc.tensor.matmul(
                                psR[:],
                                e5_sb[:, CK[k][0] : CK[k][0] + csz],
                                rec5[:],
                                start=True,
                                stop=True,
                            )
                            nc.vector.tensor_tensor(
                                out=attnT[k][0:csz, ns],
                                in0=attnT[k][0:csz, ns],
                                in1=psR[:],
                                op=ALU.mult,
                            )
                        return
                    hc, hr = h // 2, (h % 2) * 64
                    recb = recbs.pop((n, h))
                    eng = nc.gpsimd if gp_mult else nc.vector
                    eng.tensor_tensor(
                        out=attnT[hc][hr : hr + 64, ns],
                        in0=attnT[hc][hr : hr + 64, ns],
                        in1=recb[hr : hr + 64, :],
                        op=ALU.mult,
                    )

                def emit_proj_unit(t):
                    psP = apsp.tile([128, C], F32, name="psP", tag="ps320", bufs=2)
                    for k in range(3):
                        kp = _csz(k, True)
                        nc.tensor.matmul(
                            psP[:],
                            attnT[k][0:kp, t * 128 : (t + 1) * 128],
                            pw_sb[k][0:kp, :],
                            start=(k == 0),
                            stop=(k == 2),
                        )
                    o_sb = apool.tile([128, C], F32, name="o_sb", tag="o_sb", bufs=3)
                    nc.vector.tensor_copy(o_sb[:], psP[:])
                    b, i0 = t // 8, (t % 8) * 4
                    di, dj = b // 2, b % 2
                    nc.sync.dma_start(out=out_r[i0 : i0 + 4, di, :, dj, :], in_=o_sb[:])

                # deferred-work queues: urgent = psO drains (div_s1),
                # lazy = q-proj chunks, division stage 2, out-proj tiles
                urgent = []
                lazy = []

                def drain_urgent():
                    while urgent:
                        urgent.pop(0)()

                def drain_lazy():
                    if lazy:
                        lazy.pop(0)()

                for m in range(3):
                    emit_q_m(0, m)
                qcur = qhold.pop(0)

                def pair_loop(n, hc):
                    # heads 2hc (rows 0-63) and 2hc+1 (rows 64-127):
                    # concurrent row-tiled score matmuls, one batched exp
                    hA, hB = 2 * hc, 2 * hc + 1
                    psO_A = apsp.tile([65, 512], F32, name="psOA", tag="psacc", bufs=2)
                    psO_B = apsp.tile([65, 512], F32, name="psOB", tag="psacc", bufs=2)
                    p_tiles = []
                    for mc in range(8):
                        psS = apsp.tile([128, 1024], F32, name="psS", tag="psS", bufs=2)
                        nc.tensor.matmul(
                            psS[:, 0:512],
                            kT[hc][0:64, mc * 128 : (mc + 1) * 128],
                            qcur[hc][0:64, :],
                            start=True,
                            stop=True,
                        )
                        nc.tensor.matmul(
                            psS[:, 512:1024],
                            kT[hc][64:128, mc * 128 : (mc + 1) * 128],
                            qcur[hc][64:128, :],
                            start=True,
                            stop=True,
                        )
                        p_t = apool.tile(
                            [128, 1024], BF16, name="p_t", tag="p_t", bufs=4
                        )
                        nc.scalar.activation(p_t[:], psS[:], AF.Exp, scale=SCALE)
                        p_tiles.append(p_t)
                        if mc == 2:
                            drain_urgent()
                        elif mc >= 4:
                            drain_lazy()
                        if mc >= 2:
                            m = mc - 2
                            for h, off, psO in (
                                (hA, 0, psO_A),
                                (hB, 512, psO_B),
                            ):
                                nc.tensor.matmul(
                                    psO[:],
                                    v_all[:, m * 325 + h * 65 : m * 325 + (h + 1) * 65],
                                    p_tiles[m][:, off : off + 512],
                                    start=(m == 0),
                                    stop=False,
                                    skip_group_check=True,
                                )
                    for m in (6, 7):
                        for h, off, psO in ((hA, 0, psO_A), (hB, 512, psO_B)):
                            nc.tensor.matmul(
                                psO[:],
                                v_all[:, m * 325 + h * 65 : m * 325 + (h + 1) * 65],
                                p_tiles[m][:, off : off + 512],
                                start=False,
                                stop=(m == 7),
                                skip_group_check=True,
                            )
                    urgent.append(lambda: emit_division_s1(n, hA, psO_A))
                    urgent.append(lambda: emit_division_s1(n, hB, psO_B))
                    if div_mode != "pe":
                        lazy.append(lambda: emit_division_s2(n, hA))
                        lazy.append(lambda: emit_division_s2(n, hB))

                def head4_loop(n):
                    # head 4 (kT[2] rows 0-63): batch consecutive mc pairs
                    # into one [128,1024] exp; scores serial on PE.
                    psO4 = apsp.tile([65, 512], F32, name="psO4", tag="psacc", bufs=2)
                    p_tiles = []
                    for mp_ in range(4):
                        psS = apsp.tile([128, 1024], F32, name="psS4", tag="psS", bufs=2)
                        for half in range(2):
                            mc = 2 * mp_ + half
                            nc.tensor.matmul(
                                psS[:, half * 512 : (half + 1) * 512],
                                kT[2][0:64, mc * 128 : (mc + 1) * 128],
                                qcur[2][0:64, :],
                                start=True,
                                stop=True,
                            )
                        p_t = apool.tile(
                            [128, 1024], BF16, name="p_t4", tag="p_t", bufs=4
                        )
                        if dve_exp:
                            # offload head-4 exp to DVE: Schraudolph fast-exp
                            # into the bf16 bit pattern (ACT is the bottleneck
                            # engine; softmax-normalized err ~1.8% on this
                            # head only)
                            nc.vector.tensor_scalar(
                                out=p_t[:].bitcast(I16),
                                in0=psS[:],
                                scalar1=FEXP_A * SCALE,
                                scalar2=FEXP_B,
                                op0=ALU.mult,
                                op1=ALU.add,
                            )
                        else:
                            nc.scalar.activation(p_t[:], psS[:], AF.Exp, scale=SCALE)
                        p_tiles.append(p_t)
                        if mp_ == 1:
                            drain_urgent()
                        elif mp_ >= 2:
                            drain_lazy()
                            drain_lazy()
                        if mp_ >= 1:
                            m = mp_ - 1
                            for half in range(2):
                                mc = 2 * m + half
                                nc.tensor.matmul(
                                    psO4[:],
                                    v_all[:, mc * 325 + 4 * 65 : mc * 325 + 5 * 65],
                                    p_tiles[m][:, half * 512 : (half + 1) * 512],
                                    start=(mc == 0),
                                    stop=False,
                                    skip_group_check=True,
                                )
                    for half in range(2):
                        mc = 6 + half
                        nc.tensor.matmul(
                            psO4[:],
                            v_all[:, mc * 325 + 4 * 65 : mc * 325 + 5 * 65],
                            p_tiles[3][:, half * 512 : (half + 1) * 512],
                            start=False,
                            stop=(mc == 7),
                            skip_group_check=True,
                        )
                    urgent.append(lambda: emit_division_s1(n, 4, psO4))
                    if div_mode != "pe":
                        lazy.append(lambda: emit_division_s2(n, 4))

                for n in range(8):
                    pair_loop(n, 0)
                    if n < 7 and "q" not in ablate:
                        for m in range(3):
                            lazy.append(lambda n=n, m=m: emit_q_m(n + 1, m))
                    pair_loop(n, 1)
                    head4_loop(n)
                    if div_mode == "pe":
                        lazy.append(lambda n=n: emit_division_s2(n, None))
                    if "proj" not in ablate:
                        for t in range(n * 4, n * 4 + 4):
                            lazy.append(lambda t=t: emit_proj_unit(t))
                    # make sure q for n+1 is materialized before n+1 begins
                    if n < 7 and "q" not in ablate:
                        while (n + 1) not in qhold or any(
                            t is None for t in qhold[n + 1]
                        ):
                            drain_lazy()
                        qcur = qhold.pop(n + 1)
                drain_urgent()
                while lazy:
                    drain_lazy()

    nc.compile()
    return nc


_CACHE = {}


def _prep_inputs(inputs):
    x = np.ascontiguousarray(np.asarray(inputs["x"], dtype=np.float32))
    q_w = np.asarray(inputs["q_w"], np.float32)
    q_b = np.asarray(inputs["q_b"], np.float32)
    kv_w = np.asarray(inputs["kv_w"], np.float32)
    kv_b = np.asarray(inputs["kv_b"], np.float32)
    sr_w = np.asarray(inputs["sr_w"], np.float32)
    sr_b = np.asarray(inputs["sr_b"], np.float32)
    ln_g = np.asarray(inputs["ln_g"], np.float32)
    ln_b = np.asarray(inputs["ln_b"], np.float32)
    proj_w = np.asarray(inputs["proj_w"], np.float32)
    proj_b = np.asarray(inputs["proj_b"], np.float32)

    qw = np.concatenate([q_w, q_b[None, :]], axis=0)  # [321, 320]
    blocks = sr_w.reshape(4, C, C)  # HWIO -> (di*2+dj, ci, co)
    srw = np.concatenate(
        [blocks[0], sr_b[None, :], blocks[1], blocks[2], blocks[3]], axis=0
    )  # [1281, 320]
    kv_w_eff = ln_g[:, None] * kv_w
    kv_bias = ln_b @ kv_w + kv_b
    kw = np.concatenate([kv_w_eff[:, :C], kv_bias[None, :C]], axis=0)
    vw = np.concatenate([kv_w_eff[:, C:], kv_bias[None, C:]], axis=0)
    pw = np.concatenate([proj_w, proj_b[None, :]], axis=0)

    e5 = np.zeros((HEAD, C), np.float32)
    for h in range(HEAD):
        e5[h, h * HD : (h + 1) * HD] = 1.0
    import ml_dtypes

    bf = ml_dtypes.bfloat16
    consts = {
        "e5": e5.astype(bf),
        "qw": np.ascontiguousarray(qw.astype(bf)),
        "srw": np.ascontiguousarray(srw.astype(bf)),
        "kw": np.ascontiguousarray(kw.astype(bf)),
        "vw": np.ascontiguousarray(vw.astype(bf)),
        "pw": np.ascontiguousarray(pw.astype(bf)),
        "ident": np.eye(128, dtype=np.float32),
        "ones_c": np.ones((1, N), np.float32),
        "ones2": np.ones((128, 64), np.float32),
    }
    return x, consts


def kernel(**inputs) -> np.ndarray:
    H = int(np.asarray(inputs["H"]))
    W = int(np.asarray(inputs["W"]))
    assert H == 64 and W == 64, (H, W)
    x, consts = _prep_inputs(inputs)
    assert x.shape == (B, N, C)

    if "nc" not in _CACHE:
        _CACHE["nc"] = build_nc()
    nc = _CACHE["nc"]

    from concourse.bass_utils import run_bass_kernel_spmd

    in_maps = [{"x": x[i], **consts} for i in range(N_CORES)]
    res = run_bass_kernel_spmd(nc, in_maps, core_ids=list(range(N_CORES)))
    out = np.stack([res.results[i]["out"] for i in range(N_CORES)], axis=0)
    return out.astype(np.float32)

